# revision 7
# baseline (speedup 1.0000x reference)
"""Trainium2 Bass kernel for nn_BindingReadout (segment_reduce).

Computes, per batch element:
  - per-segment means of features (S=32 segments over N=8192 rows, D=256)
  - selects top MAX_OBJECTS=8 segments by count (stable sort tie-break on id)
  - projects with Linear(W, b) and applies LayerNorm(gamma, beta)

Strategy: data-parallel over batch (32 batches -> 4 per core on 8 cores).
Segment sums are computed as one-hot matmuls on the TensorEngine with a
ones-column appended to get counts for free. Top-8 selection is done with a
rank trick (rank[s] = #{s' : key[s'] > key[s]}, key = count*64 - s), which
yields a selection one-hot that is applied via matmul. No sort, no gather.
"""

import os
import sys

sys.path.insert(0, "/opt/trn_rl_repo")

import numpy as np

import concourse.bacc as bacc
import concourse.tile as tile
from concourse import mybir
from concourse.bass_utils import run_bass_kernel_spmd

# problem constants (hardcoded per contract)
B, N, D = 32, 8192, 256
S = 32            # segments per batch
M = 8             # MAX_OBJECTS
EPS = 1e-5
NCORES = 8
BPC = B // NCORES  # batches per core
P = 128            # partitions
CPB = N // P       # 64 chunks of 128 rows per batch
GRP = 8            # chunks per DMA group
NGRP = CPB // GRP  # groups per batch

# 'f32'  : full-precision fp32 matmul (4 cyc/row)
# 'f32r' : reduced-precision fp32 matmul (1 cyc/row at N>=256)
# 'hilo' : two bf16 matmuls (hi + lo split), ~fp32 precision at 2 cyc/row
MODE = os.environ.get("BASS_SEG_MODE", "hilo")
FEAT_BUFS = int(os.environ.get("BASS_FEAT_BUFS", "4"))

F32 = mybir.dt.float32
F32R = mybir.dt.float32r
BF16 = mybir.dt.bfloat16
Alu = mybir.AluOpType


def _build_nc():
    nc = bacc.Bacc(None, target_bir_lowering=False, debug=False)

    hilo = MODE == "hilo"
    if hilo:
        feat_hi = nc.dram_tensor("feat_hi", [BPC, N, D], BF16, kind="ExternalInput")
        feat_lo = nc.dram_tensor("feat_lo", [BPC, N, D], BF16, kind="ExternalInput")
        feats = [feat_hi, feat_lo]
    else:
        feats = [nc.dram_tensor("feat", [BPC, N, D], F32, kind="ExternalInput")]
    segr = nc.dram_tensor("segr", [BPC, P, CPB], F32, kind="ExternalInput")
    wt = nc.dram_tensor("wt", [D, D], F32, kind="ExternalInput")  # W.T
    brep = nc.dram_tensor("brep", [M, D], F32, kind="ExternalInput")
    grep = nc.dram_tensor("grep", [M, D], F32, kind="ExternalInput")
    prep = nc.dram_tensor("prep", [M, D], F32, kind="ExternalInput")
    iota_rep = nc.dram_tensor("iota_rep", [P, GRP * S], F32, kind="ExternalInput")
    iota8 = nc.dram_tensor("iota8", [S, M], F32, kind="ExternalInput")
    iotas = nc.dram_tensor("iotas", [S, 1], F32, kind="ExternalInput")
    ident = nc.dram_tensor("ident", [P, P], F32, kind="ExternalInput")
    out = nc.dram_tensor("out", [BPC, M, D], F32, kind="ExternalOutput")

    oh_dt = BF16 if hilo else F32
    ft_dt = BF16 if hilo else F32

    with tile.TileContext(nc) as tc:
        with (
            tc.tile_pool(name="consts", bufs=1) as cpool,
            tc.tile_pool(name="feat", bufs=FEAT_BUFS) as fpool,
            tc.tile_pool(name="oneh", bufs=3) as opool,
            tc.tile_pool(name="seg", bufs=2) as spool,
            tc.tile_pool(name="sm", bufs=2) as mpool,
            tc.tile_pool(name="pacc", bufs=2, space="PSUM") as pacc_pool,
            tc.tile_pool(name="pktr", bufs=1, space="PSUM") as pktr_pool,
            tc.tile_pool(name="pobj", bufs=1, space="PSUM") as pobj_pool,
            tc.tile_pool(name="ptr", bufs=2, space="PSUM") as ptr_pool,
            tc.tile_pool(name="pprj", bufs=1, space="PSUM") as pprj_pool,
        ):
            # constants
            wt_sb = [cpool.tile([P, D], F32, name=f"wt{h}", tag=f"wt{h}") for h in range(2)]
            for h in range(2):
                nc.sync.dma_start(wt_sb[h][:], wt[h * P:(h + 1) * P, :])
            brep_sb = cpool.tile([M, D], F32, name="brep", tag="brep")
            nc.sync.dma_start(brep_sb[:], brep[:])
            grep_sb = cpool.tile([M, D], F32, name="grep", tag="grep")
            nc.sync.dma_start(grep_sb[:], grep[:])
            prep_sb = cpool.tile([M, D], F32, name="prep", tag="prep")
            nc.sync.dma_start(prep_sb[:], prep[:])
            iota_sb = cpool.tile([P, GRP * S], F32, name="iota_rep", tag="iota_rep")
            nc.sync.dma_start(iota_sb[:], iota_rep[:])
            iota8_sb = cpool.tile([S, M], F32, name="iota8", tag="iota8")
            nc.sync.dma_start(iota8_sb[:], iota8[:])
            iotas_sb = cpool.tile([S, 1], F32, name="iotas", tag="iotas")
            nc.sync.dma_start(iotas_sb[:], iotas[:])
            id_sb = cpool.tile([P, P], F32, name="ident", tag="ident")
            nc.sync.dma_start(id_sb[:], ident[:])
            eps_sb = cpool.tile([M, 1], F32, name="epsc", tag="epsc")
            nc.vector.memset(eps_sb[:], EPS)

            iota3 = iota_sb[:].rearrange("p (g s) -> p g s", g=GRP)

            for b in range(BPC):
                # segment ids for this batch, one chunk per free column
                seg_t = spool.tile([P, CPB], F32, name="seg", tag="seg")
                nc.sync.dma_start(seg_t[:], segr[b])

                pacc = pacc_pool.tile([S, D + 1], F32, name="acc", tag="acc", space="PSUM")

                for g in range(NGRP):
                    g0 = g * GRP
                    # one-hot for GRP chunks: oh[p, g, s] = (seg[p, g] == s)
                    oh = opool.tile([P, GRP * S], oh_dt, name="oh", tag="oh")
                    oh3 = oh[:].rearrange("p (g s) -> p g s", g=GRP)
                    nc.vector.tensor_tensor(
                        out=oh3,
                        in0=seg_t[:, g0:g0 + GRP].to_broadcast([P, GRP, S]),
                        in1=iota3,
                        op=Alu.is_equal,
                    )

                    fts = []
                    for fi, fd in enumerate(feats):
                        ft = fpool.tile([P, GRP * (D + 1)], ft_dt, tag=f"ft{fi}")
                        ft3 = ft[:].rearrange("p (g x) -> p g x", g=GRP)
                        featv = fd[b].rearrange("(g p) d -> p g d", p=P)
                        nc.sync.dma_start(
                            out=ft3[:, :, 0:D], in_=featv[:, g0:g0 + GRP, :]
                        )
                        # ones column for counts (only in the hi/primary input)
                        nc.vector.memset(ft3[:, :, D:D + 1], 1.0 if fi == 0 else 0.0)
                        fts.append(ft)

                    for k in range(GRP):
                        first = g == 0 and k == 0
                        last = g == NGRP - 1 and k == GRP - 1
                        lhs = oh[:, S * k:S * (k + 1)]
                        if MODE == "f32r":
                            lhs = lhs.bitcast(F32R)
                        for fi, ft in enumerate(fts):
                            rhs = ft[:, (D + 1) * k:(D + 1) * (k + 1)]
                            if MODE == "f32r":
                                rhs = rhs.bitcast(F32R)
                            nc.tensor.matmul(
                                out=pacc[:],
                                lhsT=lhs,
                                rhs=rhs,
                                start=first and fi == 0,
                                stop=last and fi == len(fts) - 1,
                            )

                # ---- tail: means, rank, selection, projection, layernorm ----
                counts = mpool.tile([S, 1], F32, name="counts", tag="counts")
                nc.vector.tensor_copy(out=counts[:], in_=pacc[:, D:D + 1])
                cmax = mpool.tile([S, 1], F32, name="cmax", tag="cmax")
                nc.vector.tensor_scalar_max(cmax[:], counts[:], 1.0)
                recip = mpool.tile([S, 1], F32, name="recip", tag="recip")
                nc.vector.reciprocal(recip[:], cmax[:])
                mask = mpool.tile([S, 1], F32, name="mask", tag="mask")
                nc.vector.tensor_scalar(
                    mask[:], counts[:], 0.0, scalar2=None, op0=Alu.is_gt
                )
                factor = mpool.tile([S, 1], F32, name="factor", tag="factor")
                nc.vector.tensor_mul(factor[:], mask[:], recip[:])

                # means (zero for empty segments)
                means = mpool.tile([S, D], F32, name="means", tag="means")
                nc.vector.tensor_scalar(
                    means[:], pacc[:, 0:D], factor[:], scalar2=None, op0=Alu.mult
                )

                # sort key: count*64 - s  (stable desc-by-count, asc-by-id)
                kcol = mpool.tile([S, 1], F32, name="kcol", tag="kcol")
                nc.vector.scalar_tensor_tensor(
                    out=kcol[:], in0=counts[:], scalar=64.0, in1=iotas_sb[:],
                    op0=Alu.mult, op1=Alu.subtract,
                )
                # transpose-broadcast: ktr[s, s'] = key[s']
                ktr = pktr_pool.tile([S, S], F32, name="ktr", tag="ktr", space="PSUM")
                nc.tensor.transpose(
                    out=ktr[:], in_=kcol[:].to_broadcast([S, S]),
                    identity=id_sb[0:S, 0:S],
                )
                # rank[s] = sum_{s'} (key[s'] > key[s])
                gmat = mpool.tile([S, S], F32, name="gmat", tag="gmat")
                rank = mpool.tile([S, 1], F32, name="rank", tag="rank")
                nc.vector.tensor_scalar(
                    gmat[:], ktr[:], kcol[:], scalar2=0.0, op0=Alu.is_gt,
                    op1=Alu.add, accum_out=rank[:],
                )
                # selection one-hot: sel[s, m] = (rank[s] == m)
                sel = mpool.tile([S, M], F32, name="sel", tag="sel")
                nc.vector.tensor_scalar(
                    sel[:], iota8_sb[:], rank[:], scalar2=None, op0=Alu.is_equal
                )
                # objs[m, d] = sum_s sel[s, m] * means[s, d]
                pobj = pobj_pool.tile([M, D], F32, name="pobj", tag="pobj", space="PSUM")
                nc.tensor.matmul(
                    out=pobj[:], lhsT=sel[:], rhs=means[:], start=True, stop=True
                )
                objs = mpool.tile([M, D], F32, name="objs", tag="objs")
                nc.vector.tensor_copy(out=objs[:], in_=pobj[:])

                # objsT[d, m] via two PE transposes
                objsT = mpool.tile([P, 2 * M], F32, name="objsT", tag="objsT")
                for h in range(2):
                    ptr = ptr_pool.tile([P, M], F32, name="ptr", tag="ptr", space="PSUM")
                    nc.tensor.transpose(
                        out=ptr[:], in_=objs[:, h * P:(h + 1) * P],
                        identity=id_sb[0:M, 0:M],
                    )
                    nc.vector.tensor_copy(out=objsT[:, h * M:(h + 1) * M], in_=ptr[:])

                # proj[m, e] = sum_d objsT[d, m] * wt[d, e]
                pprj = pprj_pool.tile([M, D], F32, name="pprj", tag="pprj", space="PSUM")
                for h in range(2):
                    nc.tensor.matmul(
                        out=pprj[:],
                        lhsT=objsT[:, h * M:(h + 1) * M],
                        rhs=wt_sb[h][:],
                        start=h == 0,
                        stop=h == 1,
                    )

                # layernorm
                proj = mpool.tile([M, D], F32, name="proj", tag="proj")
                rowsum = mpool.tile([M, 1], F32, name="rowsum", tag="rowsum")
                nc.vector.scalar_tensor_tensor(
                    out=proj[:], in0=pprj[:], scalar=0.0, in1=brep_sb[:],
                    op0=Alu.bypass, op1=Alu.add, accum_out=rowsum[:],
                )
                mu = mpool.tile([M, 1], F32, name="mu", tag="mu")
                nc.vector.tensor_scalar_mul(mu[:], rowsum[:], 1.0 / D)
                xc = mpool.tile([M, D], F32, name="xc", tag="xc")
                nc.vector.tensor_scalar(
                    xc[:], proj[:], mu[:], scalar2=None, op0=Alu.subtract
                )
                sq = mpool.tile([M, D], F32, name="sq", tag="sq")
                varsum = mpool.tile([M, 1], F32, name="varsum", tag="varsum")
                # sq = (proj - mu) * xc = xc^2, varsum = row-sum(sq)
                nc.vector.scalar_tensor_tensor(
                    out=sq[:], in0=proj[:], scalar=mu[:], in1=xc[:],
                    op0=Alu.subtract, op1=Alu.mult, accum_out=varsum[:],
                )
                sd = mpool.tile([M, 1], F32, name="sd", tag="sd")
                nc.scalar.activation(
                    sd[:], varsum[:], mybir.ActivationFunctionType.Sqrt,
                    bias=eps_sb[:], scale=1.0 / D,
                )
                rstd = mpool.tile([M, 1], F32, name="rstd", tag="rstd")
                nc.vector.reciprocal(rstd[:], sd[:])
                y = mpool.tile([M, D], F32, name="y", tag="y")
                nc.vector.scalar_tensor_tensor(
                    out=y[:], in0=xc[:], scalar=rstd[:], in1=grep_sb[:],
                    op0=Alu.mult, op1=Alu.mult,
                )
                ob = mpool.tile([M, D], F32, name="ob", tag="ob")
                nc.vector.tensor_add(ob[:], y[:], prep_sb[:])
                nc.sync.dma_start(out=out[b], in_=ob[:])

    nc.finalize()
    return nc


_NC_CACHE = {}


def _get_nc():
    key = (MODE, FEAT_BUFS)
    if key not in _NC_CACHE:
        _NC_CACHE[key] = _build_nc()
    return _NC_CACHE[key]


def _make_in_maps(features, segment_ids, W, b, gamma, beta):
    features = np.ascontiguousarray(np.asarray(features, dtype=np.float32))
    seg = np.asarray(segment_ids).astype(np.float32)  # values in [0, 32)
    W = np.asarray(W, dtype=np.float32)
    bias = np.asarray(b, dtype=np.float32)
    gamma = np.asarray(gamma, dtype=np.float32)
    beta = np.asarray(beta, dtype=np.float32)

    # chunk-transposed segment ids: segr[b, p, c] = seg[b, c*128 + p]
    segr = np.ascontiguousarray(
        seg.reshape(B, CPB, P).transpose(0, 2, 1)
    )

    wt = np.ascontiguousarray(W.T)
    brep = np.tile(bias, (M, 1))
    grep = np.tile(gamma, (M, 1))
    prep = np.tile(beta, (M, 1))
    iota_rep = np.tile(np.arange(S, dtype=np.float32), (P, GRP))
    iota8 = np.tile(np.arange(M, dtype=np.float32), (S, 1))
    iotas = np.arange(S, dtype=np.float32).reshape(S, 1)
    ident = np.eye(P, dtype=np.float32)

    if MODE == "hilo":
        import ml_dtypes
        hi = features.astype(ml_dtypes.bfloat16)
        lo = (features - hi.astype(np.float32)).astype(ml_dtypes.bfloat16)

    in_maps = []
    for i in range(NCORES):
        sl = slice(i * BPC, (i + 1) * BPC)
        m = {
            "segr": segr[sl],
            "wt": wt, "brep": brep, "grep": grep, "prep": prep,
            "iota_rep": iota_rep, "iota8": iota8, "iotas": iotas,
            "ident": ident,
        }
        if MODE == "hilo":
            m["feat_hi"] = hi[sl]
            m["feat_lo"] = lo[sl]
        else:
            m["feat"] = features[sl]
        in_maps.append(m)
    return in_maps


def _run(features, segment_ids, W, b, gamma, beta, trace=False):
    import time
    t0 = time.time()
    nc = _get_nc()
    print(f"[kernel] nc built in {time.time() - t0:.1f}s", flush=True)
    t0 = time.time()
    in_maps = _make_in_maps(features, segment_ids, W, b, gamma, beta)
    print(f"[kernel] in_maps in {time.time() - t0:.1f}s", flush=True)
    t0 = time.time()
    res = run_bass_kernel_spmd(nc, in_maps, core_ids=list(range(NCORES)),
                               trace=trace)
    print(f"[kernel] run in {time.time() - t0:.1f}s", flush=True)
    out = np.concatenate([res.results[i]["out"] for i in range(NCORES)], axis=0)
    return out.astype(np.float32), res


def kernel(features, segment_ids, W, b, gamma, beta):
    out, _ = _run(features, segment_ids, W, b, gamma, beta, trace=False)
    return out
